# revision 17
# baseline (speedup 1.0000x reference)
"""Self-contained Trainium2 Bass kernel for nn_CrossStageAttention.

Data-parallel over batch: 16 images -> 8 NeuronCores x 2 images each.
Training-mode BatchNorm statistics are made global via two tiny AllReduces.

v1 rewrite vs baseline:
  * bf16 data path everywhere (inputs/weights converted on host); matmuls
    run bf16 at 1 cyc/row, DVE elementwise at 2x, all spills eliminated
    (xT / fusx / y stay resident in SBUF -> no DRAM round-trips).
  * softmax row-sums folded into the o-matmuls via a 257-wide augmented-V
    (ones column), killing 384 tiny PE matmuls.
  * avg-pool folded into pre-scaled w_prev (0.25x on host); max path
    compensated via 4x exp-scale and 4x(1-beta) output weight.
  * pooling / eviction work spread across DVE + Act + GpSimd engines.
  * px pipeline runs before self-attention so DVE pooling hides under
    attention matmuls; conv weights prefetched at kernel start.
  * single padded conv input buffer (34x34) with strided matmul lhsT
    access patterns instead of 3 shifted copies.

The torch "(attn@v).transpose(1,2).reshape" scramble is absorbed into the
fuse access patterns (o natural orientation): catT[i, pos=2u+v] = o[512v+i, u].
"""
import numpy as np
import ml_dtypes
from contextlib import ExitStack

import concourse.bass as bass
import concourse.tile as tile
import concourse.bacc as bacc
from concourse import mybir, masks
from concourse.bass_utils import run_bass_kernel_spmd

N_CORES = 8
IMGS = 2
C = 512
N = 1024          # query positions per image (32x32)
PC = 256
MP = 4096         # prev positions per image (64x64)
F32 = mybir.dt.float32
BF = mybir.dt.bfloat16
SCALE = 32 ** -0.5
B0_SELF = 128.0   # constant softmax-stabilization bias for self-attention
EPS = 1e-5
INV_CNT = 1.0 / (16 * 1024)
AF = mybir.ActivationFunctionType
ALU = mybir.AluOpType
X_AXIS = mybir.AxisListType.X


def build_nc():
    nc = bacc.Bacc("TRN2", target_bir_lowering=False, debug=False,
                   num_devices=N_CORES)
    x_d = nc.dram_tensor("x", [IMGS, N, C], BF, kind="ExternalInput").ap()
    px_d = nc.dram_tensor("px", [IMGS, MP, PC], BF, kind="ExternalInput").ap()
    wq_d = nc.dram_tensor("wq", [C, C], BF, kind="ExternalInput").ap()
    wp_d = nc.dram_tensor("wp", [PC, C], BF, kind="ExternalInput").ap()
    fw_d = nc.dram_tensor("fw", [2 * C, C], BF, kind="ExternalInput").ap()
    ow_d = nc.dram_tensor("ow", [9, C, C], BF, kind="ExternalInput").ap()
    g1_d = nc.dram_tensor("g1", [128, 4], F32, kind="ExternalInput").ap()
    b1_d = nc.dram_tensor("b1", [128, 4], F32, kind="ExternalInput").ap()
    g2_d = nc.dram_tensor("g2", [128, 4], F32, kind="ExternalInput").ap()
    b2_d = nc.dram_tensor("b2", [128, 4], F32, kind="ExternalInput").ap()
    pars_d = nc.dram_tensor("pars", [1, 2], F32, kind="ExternalInput").ap()
    out_d = nc.dram_tensor("out", [IMGS, N, C], F32, kind="ExternalOutput").ap()

    with tile.TileContext(nc) as tc, ExitStack() as ctx:
        const = ctx.enter_context(tc.tile_pool(name="const", bufs=1))
        keep = ctx.enter_context(tc.tile_pool(name="keep", bufs=1))
        scr = ctx.enter_context(tc.tile_pool(name="scr", bufs=5))
        ld = ctx.enter_context(tc.tile_pool(name="ld", bufs=3))
        sm = ctx.enter_context(tc.tile_pool(name="sm", bufs=10))
        ps = ctx.enter_context(tc.tile_pool(name="ps", bufs=5, space="PSUM"))
        psb = ctx.enter_context(tc.tile_pool(name="psb", bufs=2, space="PSUM"))
        dram = ctx.enter_context(tc.tile_pool(name="dram", bufs=1, space="DRAM"))

        # ------------- DRAM scratch (BN stats exchange only) -------------
        bn1_in = dram.tile([128, 8], F32, tag="bn1i")
        bn1_out = dram.tile([128, 8], F32, tag="bn1o")
        bn2_in = dram.tile([128, 8], F32, tag="bn2i")
        bn2_out = dram.tile([128, 8], F32, tag="bn2o")

        # ------------- constants / params -------------
        identF = const.tile([128, 128], F32, tag="identF")
        masks.make_identity(nc, identF[:])
        identB = const.tile([128, 128], BF, tag="identB")
        nc.vector.tensor_copy(identB[:], identF[:])
        onesF = const.tile([128, 1], F32, tag="onesF")
        nc.gpsimd.memset(onesF[:], 1.0)
        ones_bf = const.tile([128, 1], BF, tag="onesbf")
        nc.vector.tensor_copy(ones_bf[:], onesF[:])
        b0s = const.tile([128, 1], F32, tag="b0s")
        nc.gpsimd.memset(b0s[:], -B0_SELF)
        eps_t = const.tile([128, 1], F32, tag="eps")
        nc.gpsimd.memset(eps_t[:], EPS)
        g1_s = const.tile([128, 4], F32, tag="g1")
        b1_s = const.tile([128, 4], F32, tag="b1")
        pars_s = const.tile([1, 2], F32, tag="pars")
        pars_bc = const.tile([128, 2], F32, tag="parsbc")
        s1acc = const.tile([128, 4, 4], F32, tag="s1acc")
        ss1acc = const.tile([128, 4, 4], F32, tag="ss1acc")
        s1v = const.tile([128, 4], F32, tag="s1v")
        t1v = const.tile([128, 4], F32, tag="t1v")

        # small params via gpsimd SWDGE; big weights too (keeps the SP
        # queue free for x/px streaming and Act queue free for evictions)
        nc.gpsimd.dma_start(g1_s[:], g1_d)
        nc.gpsimd.dma_start(b1_s[:], b1_d)
        nc.gpsimd.dma_start(pars_s[:], pars_d)
        nc.gpsimd.partition_broadcast(pars_bc[:], pars_s[:])

        wq_s = const.tile([128, 4, C], BF, tag="wq")
        wp_s = const.tile([128, 2, C], BF, tag="wp")
        fw_s = const.tile([128, 8, C], BF, tag="fw")
        ow_s = const.tile([128, 9, 4, C], BF, tag="ow")
        nc.gpsimd.dma_start(wq_s[:], wq_d.rearrange("(ic p) c -> p ic c", p=128))
        nc.gpsimd.dma_start(wp_s[:], wp_d.rearrange("(ic p) c -> p ic c", p=128))

        # persistent per-image tensors (live into the conv phase)
        xT_t = [keep.tile([128, 4, N], BF, tag=f"xT{i}", name=f"xT{i}")
                for i in range(IMGS)]
        fsb_t = [keep.tile([128, 4, N], BF, tag=f"fsb{i}", name=f"fsb{i}")
                 for i in range(IMGS)]
        y_s = keep.tile([128, IMGS, 4, 2, C], BF, tag="ys")

        # =================== attention scope ===================
        with tc.tile_pool(name="attn", bufs=1) as ap_:
            for img in range(IMGS):
                xT = xT_t[img]
                qT = ap_.tile([128, 4, N], BF, tag="qT", name="qT")
                avgT = ap_.tile([128, 4, N], BF, tag="avgT", name="avgT")
                maxT = ap_.tile([128, 4, N], BF, tag="maxT", name="maxT")
                xnow_t = ap_.tile([128, 8, C], BF, tag="xnow", name="xnow")
                xprev_t = ap_.tile([128, 8, C], BF, tag="xprev", name="xprev")

                # ---- x load + PE transpose -> xT (bf16, resident)
                for nt in range(8):
                    xl = ld.tile([128, C], BF, tag="xl", name="xl")
                    nc.sync.dma_start(xl[:], x_d[img, 128 * nt:128 * nt + 128, :])
                    pt = psb.tile([128, 512], BF, tag="pt", name="ptx")
                    for ci in range(4):
                        nc.tensor.transpose(pt[:, 128 * ci:128 * ci + 128],
                                            xl[:, 128 * ci:128 * ci + 128],
                                            identB[:])
                    dst = xT[:, :, 128 * nt:128 * nt + 128]
                    src = pt[:].rearrange("p (ci n) -> p ci n", ci=4)
                    if nt % 2 == 0:
                        nc.vector.tensor_copy(dst, src)
                    else:
                        nc.scalar.copy(dst, src)

                # ---- qT projection (wq lhsT x xT)
                for ci in range(4):
                    for nh in range(2):
                        qp = ps.tile([128, 512], F32, tag="b", name="qp")
                        for ic in range(4):
                            nc.tensor.matmul(
                                qp[:], wq_s[:, ic, 128 * ci:128 * ci + 128],
                                xT[:, ic, 512 * nh:512 * nh + 512],
                                start=(ic == 0), stop=(ic == 3))
                        nc.scalar.copy(qT[:, ci, 512 * nh:512 * nh + 512],
                                       qp[:])

                # ---- px pipeline: transpose, project, pool (before
                #      self-attn so DVE pooling hides under attention mms)
                for ch in range(8):
                    pxc = ap_.tile([128, 2, 512], BF, tag="pxc", bufs=2,
                                   name="pxc")
                    for kk in range(4):
                        pl = ld.tile([128, PC], BF, tag="pl", bufs=4,
                                     name="pl")
                        nc.sync.dma_start(
                            pl[:],
                            px_d[img, 512 * ch + 128 * kk:
                                 512 * ch + 128 * kk + 128, :])
                        ptp = psb.tile([128, 512], BF, tag="pt", name="ptp")
                        for pc in range(2):
                            nc.tensor.transpose(
                                ptp[:, 128 * pc:128 * pc + 128],
                                pl[:, 128 * pc:128 * pc + 128], identB[:])
                        dst = pxc[:, :, 128 * kk:128 * kk + 128]
                        src = ptp[:, 0:256].rearrange("p (pc n) -> p pc n",
                                                      pc=2)
                        nc.vector.tensor_copy(dst, src)
                    for ci in range(4):
                        pq = ps.tile([128, 512], F32, tag="b", name="pq")
                        for pc in range(2):
                            nc.tensor.matmul(
                                pq[:], wp_s[:, pc, 128 * ci:128 * ci + 128],
                                pxc[:, pc, :],
                                start=(pc == 0), stop=(pc == 1))
                        # fast Act eviction frees the PSUM bank; pooling
                        # runs on the bf16 SBUF copy (DVE 2x + GpSimd)
                        pqs = scr.tile([128, 512], BF, tag="pqs", bufs=4,
                                       name="pqs")
                        nc.scalar.copy(pqs[:], pq[:])
                        v = pqs[:].rearrange("p (i a j b) -> p i a j b",
                                             i=4, a=2, j=32, b=2)
                        mx1 = scr.tile([128, 256], BF, tag="p256", bufs=4,
                                       name="mx1")
                        mv = mx1[:].rearrange("p (i a j) -> p i a j",
                                              i=4, a=2)
                        nc.vector.tensor_tensor(mv, v[:, :, :, :, 0],
                                                v[:, :, :, :, 1], op=ALU.max)
                        nc.vector.tensor_tensor(
                            maxT[:, ci, 128 * ch:128 * ch + 128]
                            .rearrange("p (i j) -> p i j", j=32),
                            mv[:, :, 0, :], mv[:, :, 1, :], op=ALU.max)
                        # avg: wp pre-scaled 0.25 so plain sums suffice
                        av1 = scr.tile([128, 256], BF, tag="p256", bufs=4,
                                       name="av1")
                        avv = av1[:].rearrange("p (i a j) -> p i a j",
                                               i=4, a=2)
                        nc.vector.tensor_tensor(avv, v[:, :, :, :, 0],
                                                v[:, :, :, :, 1], op=ALU.add)
                        nc.vector.tensor_tensor(
                            avgT[:, ci, 128 * ch:128 * ch + 128]
                            .rearrange("p (i j) -> p i j", j=32),
                            avv[:, :, 0, :], avv[:, :, 1, :], op=ALU.add)

                # ---- augmented V in natural orientation (ones col at
                #      256 and 513 -> row-sums fall out of the o-matmuls)
                def vaug_ones(va):
                    nc.gpsimd.memset(va[:, :, 256:257], 1.0)
                    nc.gpsimd.memset(va[:, :, 513:514], 1.0)

                def evict_vaug(va, mi, src, eng):
                    if eng == "act":
                        nc.scalar.copy(va[:, mi, 0:256], src[:, 0:256])
                        nc.scalar.copy(va[:, mi, 257:513], src[:, 256:512])
                    else:
                        nc.vector.tensor_copy(va[:, mi, 0:256], src[:, 0:256])
                        nc.vector.tensor_copy(va[:, mi, 257:513],
                                              src[:, 256:512])

                # self-attention V = qkv natural, via qT transposes
                vaug = ap_.tile([128, 8, 514], BF, tag="vaug", name="vaug_s")
                vaug_ones(vaug)
                for mi in range(8):
                    pts = psb.tile([128, 512], BF, tag="pt", name="pts")
                    for ci in range(4):
                        nc.tensor.transpose(
                            pts[:, 128 * ci:128 * ci + 128],
                            qT[:, ci, 128 * mi:128 * mi + 128], identB[:])
                    evict_vaug(vaug, mi, pts[:], "act" if mi % 2 else "vec")

                def do_attn(kind, kvT, va):
                    bias = b0s[:] if kind == "self" else 0.0
                    scale = SCALE * (4.0 if kind == "max" else 1.0)
                    for nh in range(2):
                        eas = []
                        for mi in range(8):
                            lg = ps.tile([128, 512], F32, tag="b", name="lg")
                            for ci in range(4):
                                nc.tensor.matmul(
                                    lg[:],
                                    kvT[:, ci, 128 * mi:128 * mi + 128],
                                    qT[:, ci, 512 * nh:512 * nh + 512],
                                    start=(ci == 0), stop=(ci == 3))
                            ea = scr.tile([128, 512], BF, tag="ea", bufs=9,
                                          name="ea")
                            nc.scalar.activation(ea[:], lg[:], AF.Exp,
                                                 bias=bias, scale=scale)
                            eas.append(ea)
                        for np2 in range(2):
                            for k in range(2):
                                oa = ps.tile([128, 512], F32, tag="b",
                                             name="oa")
                                ob = ps.tile([128, 512], F32, tag="b",
                                             name="ob")
                                for mi in range(8):
                                    lhsT = eas[mi][:, 128 * (2 * np2 + k):
                                                   128 * (2 * np2 + k) + 128]
                                    nc.tensor.matmul(oa[:, 0:257], lhsT,
                                                     va[:, mi, 0:257],
                                                     start=(mi == 0),
                                                     stop=(mi == 7))
                                    nc.tensor.matmul(ob[:, 0:257], lhsT,
                                                     va[:, mi, 257:514],
                                                     start=(mi == 0),
                                                     stop=(mi == 7))
                                nck = 4 * nh + 2 * np2 + k
                                rec = sm.tile([128, 1], F32, name="rec")
                                nc.vector.reciprocal(rec[:], oa[:, 256:257])
                                if kind == "self":
                                    w = rec
                                elif kind == "avg":
                                    w = sm.tile([128, 1], F32, name="bw")
                                    nc.vector.tensor_tensor(
                                        w[:], rec[:], pars_bc[:, 0:1],
                                        op=ALU.mult)
                                else:
                                    w = sm.tile([128, 1], F32, name="bw")
                                    nc.vector.tensor_tensor(
                                        w[:], rec[:], pars_bc[:, 1:2],
                                        op=ALU.mult)
                                if kind == "max":
                                    t_ = scr.tile([128, 512], BF, tag="s",
                                                  name="mx")
                                    nc.scalar.mul(t_[:, 0:256],
                                                  oa[:, 0:256], w[:])
                                    nc.vector.tensor_scalar_mul(
                                        t_[:, 256:512], ob[:, 0:256], w[:])
                                    nc.vector.tensor_tensor(
                                        xprev_t[:, nck, :],
                                        xprev_t[:, nck, :], t_[:],
                                        op=ALU.add)
                                else:
                                    dstt = (xnow_t if kind == "self"
                                            else xprev_t)
                                    nc.scalar.mul(dstt[:, nck, 0:256],
                                                  oa[:, 0:256], w[:])
                                    nc.vector.tensor_scalar_mul(
                                        dstt[:, nck, 256:512],
                                        ob[:, 0:256], w[:])

                do_attn("self", qT, vaug)
                if img == 0:
                    # big weights stream in behind the px loads
                    nc.gpsimd.dma_start(
                        fw_s[:], fw_d.rearrange("(ic p) o -> p ic o", p=128))
                    nc.gpsimd.dma_start(
                        ow_s[:],
                        ow_d.rearrange("t (ic p) o -> p t ic o", p=128))

                # avg attention: rebuild vaug by transposing avgT
                vaug = ap_.tile([128, 8, 514], BF, tag="vaug", name="vaug_a")
                vaug_ones(vaug)
                for mi in range(8):
                    ptn = psb.tile([128, 512], BF, tag="pt", name="ptn")
                    for ci in range(4):
                        nc.tensor.transpose(
                            ptn[:, 128 * ci:128 * ci + 128],
                            avgT[:, ci, 128 * mi:128 * mi + 128], identB[:])
                    evict_vaug(vaug, mi, ptn[:], "act" if mi % 2 else "vec")
                do_attn("avg", avgT, vaug)

                # max attention
                vaug = ap_.tile([128, 8, 514], BF, tag="vaug", name="vaug_m")
                vaug_ones(vaug)
                for mi in range(8):
                    ptn = psb.tile([128, 512], BF, tag="pt", name="ptm")
                    for ci in range(4):
                        nc.tensor.transpose(
                            ptn[:, 128 * ci:128 * ci + 128],
                            maxT[:, ci, 128 * mi:128 * mi + 128], identB[:])
                    evict_vaug(vaug, mi, ptn[:], "act" if mi % 2 else "vec")
                do_attn("max", maxT, vaug)

                # ---- fuse matmul + BN1 partial stats; fusx resident,
                #      stored position-interleaved: fsb[:, oi, 2u+v]
                fsb = fsb_t[img]
                for oi in range(4):
                    for v in range(2):
                        fp = ps.tile([128, 512], F32, tag="b", name="fp")
                        for ii in range(8):
                            rhs = (xnow_t[:, 4 * v + ii, :] if ii < 4
                                   else xprev_t[:, 4 * v + (ii - 4), :])
                            nc.tensor.matmul(
                                fp[:], fw_s[:, ii, 128 * oi:128 * oi + 128],
                                rhs, start=(ii == 0), stop=(ii == 7))
                        slot = 2 * img + v
                        dst = (fsb[:, oi, :]
                               .rearrange("p (u two) -> p u two", two=2)
                               [:, :, v])
                        nc.scalar.activation(
                            dst, fp[:], AF.Copy,
                            accum_out=s1acc[:, oi, slot:slot + 1])
                        sqt = scr.tile([128, 512], BF, tag="s", name="sqt")
                        nc.scalar.activation(
                            sqt[:], fp[:], AF.Square,
                            accum_out=ss1acc[:, oi, slot:slot + 1])

        # =================== BN1 global stats ===================
        sum1 = sm.tile([128, 4], F32, name="sum1")
        ssq1 = sm.tile([128, 4], F32, name="ssq1")
        nc.vector.tensor_reduce(sum1[:], s1acc[:], axis=X_AXIS, op=ALU.add)
        nc.vector.tensor_reduce(ssq1[:], ss1acc[:], axis=X_AXIS, op=ALU.add)
        nc.gpsimd.dma_start(bn1_in[:, 0:4], sum1[:])
        nc.gpsimd.dma_start(bn1_in[:, 4:8], ssq1[:])
        nc.gpsimd.collective_compute(
            "AllReduce", ALU.add, replica_groups=[list(range(N_CORES))],
            ins=[bn1_in.opt()], outs=[bn1_out.opt()])

        # =================== conv scope ===================
        with tc.tile_pool(name="conv", bufs=1) as cp_:
            # 3 column-shifted, vertically padded buffers per image
            # (matmul operands need a single free dim):
            #   x2s[:, d, ci, r*32 + w] = x2[r-1, w+d-1]  (0 outside)
            # Pre-filled with the residual x^T DURING the BN1 AllReduce;
            # the BN1-dependent relu term is added in afterwards.
            x2ss = []
            strips = []
            for img in range(IMGS):
                x2s = cp_.tile([128, 3, 4, 1088], BF, tag=f"x2s{img}",
                               name=f"x2s{img}")
                x2ss.append(x2s)
                for ci in range(4):
                    ctr = x2s[:, 1, ci, :]
                    nc.gpsimd.memset(ctr[0:128, 0:32], 0.0)
                    nc.gpsimd.memset(ctr[0:128, 1056:1088], 0.0)
                    nc.vector.tensor_copy(ctr[0:128, 32:1056],
                                          xT_t[img][:, ci, :])
                    nc.vector.tensor_copy(x2s[:, 0, ci, 1:1088],
                                          ctr[0:128, 0:1087])
                    s0 = (x2s[:, 0, ci, :]
                          .rearrange("p (r w) -> p r w", w=32)[:, :, 0])
                    nc.gpsimd.memset(s0, 0.0)
                    nc.scalar.copy(x2s[:, 2, ci, 0:1087],
                                   ctr[0:128, 1:1088])
                    s2_ = (x2s[:, 2, ci, :]
                           .rearrange("p (r w) -> p r w", w=32)[:, :, 31])
                    nc.gpsimd.memset(s2_, 0.0)
                    strips.append((s0, s2_))

            # ---- BN1 math (waits on the collective)
            allst = sm.tile([128, 8], F32, name="allst")
            nc.sync.dma_start(allst[:], bn1_out[:])
            mean1 = sm.tile([128, 4], F32, name="mean1")
            tA = sm.tile([128, 4], F32, name="tA")
            tB = sm.tile([128, 4], F32, name="tB")
            nc.scalar.mul(mean1[:], allst[:, 0:4], INV_CNT)
            nc.scalar.mul(tA[:], allst[:, 4:8], INV_CNT)
            nc.scalar.square(tB[:], mean1[:])
            nc.vector.tensor_tensor(tA[:], tA[:], tB[:], op=ALU.subtract)
            nc.scalar.activation(tA[:], tA[:], AF.Sqrt, bias=eps_t[:])
            nc.vector.reciprocal(tA[:], tA[:])
            nc.vector.tensor_tensor(s1v[:], g1_s[:], tA[:], op=ALU.mult)
            nc.vector.tensor_tensor(tB[:], mean1[:], s1v[:], op=ALU.mult)
            nc.vector.tensor_tensor(t1v[:], b1_s[:], tB[:], op=ALU.subtract)

            # ---- add the BN1+relu fuse term into all 3 shifted buffers
            for img in range(IMGS):
                x2s = x2ss[img]
                for ci in range(4):
                    rt = scr.tile([128, N], BF, tag="rt", bufs=2, name="rt")
                    nc.scalar.activation(rt[:], fsb_t[img][:, ci, :],
                                         AF.Relu, bias=t1v[:, ci:ci + 1],
                                         scale=s1v[:, ci:ci + 1])
                    c1 = x2s[:, 1, ci, 32:1056]
                    nc.vector.tensor_tensor(c1, c1, rt[:], op=ALU.add)
                    c0 = x2s[:, 0, ci, 33:1057]
                    nc.vector.tensor_tensor(c0, c0, rt[:], op=ALU.add)
                    c2 = x2s[:, 2, ci, 32:1055]
                    nc.vector.tensor_tensor(c2, c2, rt[:, 1:1024],
                                            op=ALU.add)
                    s0, s2_ = strips[4 * img + ci]
                    nc.gpsimd.memset(s0, 0.0)
                    nc.gpsimd.memset(s2_, 0.0)

            # ---- conv 3x3, transposed output yT[oc, pos]; BN2 stats
            #      fall out of the Act accumulator on eviction
            s2acc = cp_.tile([128, 4, 4], F32, tag="s2acc", name="s2acc")
            ss2acc = cp_.tile([128, 4, 4], F32, tag="ss2acc", name="ss2acc")
            for img in range(IMGS):
                for ocb in range(4):
                    for pt in range(2):
                        yp = ps.tile([128, 512], F32, tag="b", name="yp")
                        k = 0
                        for tap in range(9):
                            dh, dw = tap // 3, tap % 3
                            for ci in range(4):
                                rhs = x2ss[img][:, dw, ci,
                                                32 * (16 * pt + dh):
                                                32 * (16 * pt + dh) + 512]
                                nc.tensor.matmul(
                                    yp[:],
                                    ow_s[:, tap, ci,
                                         128 * ocb:128 * ocb + 128],
                                    rhs, start=(k == 0), stop=(k == 35))
                                k += 1
                        slot = 2 * img + pt
                        nc.scalar.activation(
                            y_s[:, img, ocb, pt, :], yp[:], AF.Copy,
                            accum_out=s2acc[:, ocb, slot:slot + 1])
                        ysq = scr.tile([128, 512], BF, tag="s", name="ysq")
                        nc.scalar.activation(
                            ysq[:], yp[:], AF.Square,
                            accum_out=ss2acc[:, ocb, slot:slot + 1])

            # ---- BN2 global stats + math ([128,4] mirror of BN1)
            sum2 = sm.tile([128, 4], F32, name="sum2")
            ssq2 = sm.tile([128, 4], F32, name="ssq2")
            nc.vector.tensor_reduce(sum2[:], s2acc[:], axis=X_AXIS,
                                    op=ALU.add)
            nc.vector.tensor_reduce(ssq2[:], ss2acc[:], axis=X_AXIS,
                                    op=ALU.add)
            nc.gpsimd.dma_start(bn2_in[:, 0:4], sum2[:])
            nc.gpsimd.dma_start(bn2_in[:, 4:8], ssq2[:])
            nc.gpsimd.collective_compute(
                "AllReduce", ALU.add, replica_groups=[list(range(N_CORES))],
                ins=[bn2_in.opt()], outs=[bn2_out.opt()])
            g2_s = cp_.tile([128, 4], F32, tag="g2s", name="g2s")
            b2_s = cp_.tile([128, 4], F32, tag="b2s", name="b2s")
            nc.gpsimd.dma_start(g2_s[:], g2_d)
            nc.gpsimd.dma_start(b2_s[:], b2_d)
            allst2 = sm.tile([128, 8], F32, name="allst2")
            nc.sync.dma_start(allst2[:], bn2_out[:])
            mean2 = sm.tile([128, 4], F32, name="mean2")
            uA = sm.tile([128, 4], F32, name="uA")
            uB = sm.tile([128, 4], F32, name="uB")
            s2v = cp_.tile([128, 4], F32, tag="s2v", name="s2v")
            t2v = cp_.tile([128, 4], F32, tag="t2v", name="t2v")
            nc.scalar.mul(mean2[:], allst2[:, 0:4], INV_CNT)
            nc.scalar.mul(uA[:], allst2[:, 4:8], INV_CNT)
            nc.scalar.square(uB[:], mean2[:])
            nc.vector.tensor_tensor(uA[:], uA[:], uB[:], op=ALU.subtract)
            nc.scalar.activation(uA[:], uA[:], AF.Sqrt, bias=eps_t[:])
            nc.vector.reciprocal(uA[:], uA[:])
            nc.vector.tensor_tensor(s2v[:], g2_s[:], uA[:], op=ALU.mult)
            nc.vector.tensor_tensor(uB[:], mean2[:], s2v[:], op=ALU.mult)
            nc.vector.tensor_tensor(t2v[:], b2_s[:], uB[:], op=ALU.subtract)

            # ---- BN2 apply (per-partition scale/bias on Act), PE
            #      transpose back to [n, c], store
            for img in range(IMGS):
                for pt in range(2):
                    wb = cp_.tile([128, 4, 512], BF, tag="wb", bufs=2,
                                  name="wb")
                    for ocb in range(4):
                        nc.scalar.activation(
                            wb[:, ocb, :], y_s[:, img, ocb, pt, :], AF.Relu,
                            bias=t2v[:, ocb:ocb + 1],
                            scale=s2v[:, ocb:ocb + 1])
                    for nb in range(4):
                        ptt = psb.tile([128, 512], BF, tag="pt", name="ptt")
                        for ocb in range(4):
                            nc.tensor.transpose(
                                ptt[:, 128 * ocb:128 * ocb + 128],
                                wb[:, ocb, 128 * nb:128 * nb + 128],
                                identB[:])
                        w1 = scr.tile([128, 512], F32, tag="w1", bufs=2,
                                      name="w1")
                        nc.vector.tensor_copy(w1[:], ptt[:])
                        base = 512 * pt + 128 * nb
                        nc.sync.dma_start(out_d[img, base:base + 128, :],
                                          w1[:])

    nc.compile()
    return nc


_STATE = {}


def _get_nc():
    if "nc" not in _STATE:
        _STATE["nc"] = build_nc()
    return _STATE["nc"]


def make_in_maps(x, prevx, w_prev_qkv, w_qkv, fuse_w, fuse_b, bn1_g, bn1_b,
                 out_w, out_b, bn2_g, bn2_b, gamma, beta):
    f = np.float32
    bf = ml_dtypes.bfloat16
    wq = np.ascontiguousarray(np.asarray(w_qkv, f).T.astype(bf))
    # 0.25x: folds the avg-pool normalization into the projection; the
    # max path is compensated by 4x exp-scale and 4x(1-beta) weight.
    wp = np.ascontiguousarray((0.25 * np.asarray(w_prev_qkv, f).T).astype(bf))
    fw = np.ascontiguousarray(np.asarray(fuse_w, f).astype(bf))
    ow = np.ascontiguousarray(np.asarray(out_w, f).reshape(9, C, C).astype(bf))
    g = float(np.asarray(gamma, f).reshape(-1)[0])
    g1 = np.ascontiguousarray((g * np.asarray(bn1_g, f)).reshape(4, 128).T)
    b1 = np.ascontiguousarray((g * np.asarray(bn1_b, f)).reshape(4, 128).T)
    g2 = np.ascontiguousarray(np.asarray(bn2_g, f).reshape(4, 128).T)
    b2 = np.ascontiguousarray(np.asarray(bn2_b, f).reshape(4, 128).T)
    bt = float(np.asarray(beta, f).reshape(-1)[0])
    pars = np.array([[bt, 4.0 * (1.0 - bt)]], f)
    xf = np.asarray(x, f).reshape(16, N, C).astype(bf)
    pxf = np.asarray(prevx, f).reshape(16, MP, PC).astype(bf)
    maps = []
    for c in range(N_CORES):
        maps.append({
            "x": np.ascontiguousarray(xf[2 * c:2 * c + 2]),
            "px": np.ascontiguousarray(pxf[2 * c:2 * c + 2]),
            "wq": wq, "wp": wp, "fw": fw, "ow": ow,
            "g1": g1, "b1": b1, "g2": g2, "b2": b2, "pars": pars,
        })
    return maps


def kernel(**inputs):
    nc = _get_nc()
    maps = make_in_maps(**inputs)
    res = run_bass_kernel_spmd(nc, maps, list(range(N_CORES)))
    out = np.concatenate([res.results[c]["out"] for c in range(N_CORES)],
                         axis=0)
    return out.reshape(16, 32, 32, C).astype(np.float32)
